# revision 33
# baseline (speedup 1.0000x reference)
"""DeformMCALayer Trainium2 kernel: 8-way data-parallel over batch.

kernel(**inputs) takes the FULL inputs (x [8,256,64,64], offset_w [18,256,3,3],
deform_w [256,256,3,3]) and returns the FULL output [8,256,64,64] (float32).

Per core (one image):
  1. offset conv (3x3, bf16 matmuls, PSUM-accumulated)
  2. PE-transpose offsets to pixel-partition layout; DVE index/bilinear-weight
     math (floor via 1.5*2^23 magic, clip, validity masks)
  3. x transposed to pixel-major bf16; a 4-corner-packed DRAM table xt2 is
     built (row p = [x[p], x[p+1], x[p+64], x[p+65]]) so ONE SWDGE gather
     descriptor fetches all 4 bilinear corners (2KB) per (tap, pixel)
  4. bilinear weighting: one broadcast tensor_tensor mult (stride-0 weight
     AP over channels) + two pair-sum adds per (tap, quarter)
  5. PE-transpose patches to (c,k)-partition tiles; bf16 matmul vs deform_w
     (contraction 2304) accumulating f32 in PSUM
  6. channel attention (mean + unbiased std -> sigmoid) fused on ACT reading
     PSUM; final scale + store
"""
import sys
sys.path.insert(0, "/opt/trn_rl_repo")
import numpy as np
import ml_dtypes

import concourse.bacc as bacc
import concourse.mybir as mybir
from concourse.tile import TileContext
from concourse.ap import AP

F32 = mybir.dt.float32
BF16 = mybir.dt.bfloat16
I16 = mybir.dt.int16
OP = mybir.AluOpType
AF = mybir.ActivationFunctionType

H = W = 64
HW = 4096
K = 9
NQT = 4
QPIX = 1024
PADROWS = 65  # lead pad so shifted copies never write negative rows
XT2ROWS = PADROWS + HW + PADROWS  # lead pad + 4096 pixel rows + zero tail
N_CORES = 8
MAGIC = float(3 * 2 ** 22)  # 1.5*2^23 round-to-int magic (|x| < 2^22)


def _mk(ap_or_handle, extra_offset, dims):
    if isinstance(ap_or_handle, AP):
        t, off = ap_or_handle.tensor, ap_or_handle.offset
    else:
        a = ap_or_handle.ap()
        t, off = a.tensor, a.offset
    return AP(t, off + extra_offset, [list(d) for d in dims])


def build_program(repeat=1, stage=4):
    nc = bacc.Bacc("TRN2", target_bir_lowering=False, debug=False, num_devices=1,
                   num_swdge_queues=2)

    x_d = nc.dram_tensor("x", [256, HW], F32, kind="ExternalInput")
    offw_d = nc.dram_tensor("offw", [128, 2, K, 18], BF16, kind="ExternalInput")
    w2_d = nc.dram_tensor("w2", [128, 18, 256], BF16, kind="ExternalInput")
    basyx_d = nc.dram_tensor("basyx", [128, 32, 18], F32, kind="ExternalInput")
    idf_d = nc.dram_tensor("idf", [128, 128], F32, kind="ExternalInput")
    idb_d = nc.dram_tensor("idb", [128, 128], BF16, kind="ExternalInput")

    xt2_d = nc.dram_tensor("xt2", [XT2ROWS, 1024], BF16, kind="Internal")
    y_d = nc.dram_tensor("y", [256, HW], F32, kind="ExternalOutput")

    with TileContext(nc) as tc:
        for _rep in range(repeat):
            with tc.tile_pool(name="const", bufs=1) as cpool:
                w2_sb = cpool.tile([128, 18, 256], BF16)
                nc.sync.dma_start(w2_sb[:], w2_d[:])
                offw_sb = cpool.tile([128, 2, K, 18], BF16)
                nc.sync.dma_start(offw_sb[:], offw_d[:])
                basyx_sb = cpool.tile([128, 32, 18], F32)
                nc.sync.dma_start(basyx_sb[:], basyx_d[:])
                idf_sb = cpool.tile([128, 128], F32)
                nc.sync.dma_start(idf_sb[:], idf_d[:])
                idb_sb = cpool.tile([128, 128], BF16)
                nc.sync.dma_start(idb_sb[:], idb_d[:])

                # zero-fill xt2 regions that the shifted copies cannot cover
                # (content pixels beyond 4095 must read as 0.0)
                zt = cpool.tile([128, 256], BF16)
                nc.vector.memset(zt[:], 0.0)
                nc.sync.dma_start(
                    _mk(xt2_d, (PADROWS - 1 + HW) * 1024 + 256, [[1024, 1], [1, 256]]),
                    zt[0:1, :])
                nc.sync.dma_start(
                    _mk(xt2_d, (PADROWS - 64 + HW) * 1024 + 512, [[1024, 64], [1, 256]]),
                    zt[0:64, :])
                nc.sync.dma_start(
                    _mk(xt2_d, (PADROWS - 65 + HW) * 1024 + 768, [[1024, 65], [1, 256]]),
                    zt[0:65, :])

                offT = cpool.tile([128, 32, 18], F32)
                w4c = cpool.tile([128, NQT, K, 8, 4], BF16)
                idxw16 = cpool.tile([16, K, NQT, 64], I16)
                idxw = cpool.tile([128, K, NQT, 64], I16)

                # ---------------- boot: conv + transposes ----------------
                with tc.tile_pool(name="boot", bufs=1) as bpool, \
                     tc.tile_pool(name="psconv", bufs=2, space="PSUM") as psconv, \
                     tc.tile_pool(name="pstp", bufs=2, space="PSUM") as pstp:
                    # fast contiguous HWDGE loads (f32), then DVE cast to bf16
                    x_f = bpool.tile([128, 2, HW], F32)
                    for cb in range(2):
                        src = _mk(x_d, cb * 128 * HW, [[HW, 128], [1, HW]])
                        nc.sync.dma_start(x_f[:, cb, :], src)
                    x_pad = bpool.tile([128, 2, 66, 66], BF16)
                    nc.vector.memset(x_pad[:, :, 0, :], 0.0)
                    nc.vector.memset(x_pad[:, :, 65, :], 0.0)
                    nc.vector.memset(x_pad[:, :, 1:65, 0:1], 0.0)
                    nc.vector.memset(x_pad[:, :, 1:65, 65:66], 0.0)
                    for cb in range(2):
                        nc.vector.tensor_copy(x_pad[:, cb, 1:65, 1:65],
                                              x_f[:, cb, :])

                    off_sb = bpool.tile([18, HW], F32)
                    for chk in range(8):
                        ps_conv = psconv.tile([18, 512], F32, tag="conv")
                        r0 = chk * 8
                        idx = 0
                        for cb in range(2):
                            for k in range(K):
                                ky, kx = k // 3, k % 3
                                rhs = x_pad[:, cb, r0 + ky: r0 + ky + 8, kx: kx + 64]
                                nc.tensor.matmul(
                                    ps_conv[:], offw_sb[:, cb, k, :], rhs,
                                    start=(idx == 0), stop=(idx == 17))
                                idx += 1
                        nc.scalar.copy(off_sb[:, chk * 512:(chk + 1) * 512], ps_conv[:])

                    for b in range(32):
                        ps_t = pstp.tile([128, 18], F32, tag="tp18")
                        nc.tensor.transpose(ps_t[:], off_sb[:, b * 128:(b + 1) * 128],
                                            idf_sb[0:18, 0:18])
                        nc.scalar.copy(offT[:, b, :], ps_t[:])

                    x_unpad = bpool.tile([128, 2, HW], BF16)
                    for cb in range(2):
                        nc.vector.tensor_copy(x_unpad[:, cb, :], x_f[:, cb, :])
                    xT_sb = bpool.tile([128, 32, 256], BF16)
                    for cb in range(2):
                        for b in range(32):
                            ps_x = pstp.tile([128, 128], BF16, tag="tpx")
                            in_ap = x_unpad[:, cb, b * 128:(b + 1) * 128]
                            nc.tensor.transpose(ps_x[:], in_ap, idb_sb[:])
                            nc.scalar.copy(xT_sb[:, b, cb * 128:(cb + 1) * 128], ps_x[:])
                    # 4-corner pack: xt2 row (PADROWS+p) = [x[p], x[p+1],
                    # x[p+64], x[p+65]] via 4 shifted SBUF->DRAM copies
                    for j, shift in enumerate([0, 1, 64, 65]):
                        dst = _mk(xt2_d, (PADROWS - shift) * 1024 + j * 256,
                                  [[1024, 128], [128 * 1024, 32], [1, 256]])
                        nc.sync.dma_start(dst, xT_sb[:])

                # ---------------- index & weight math ----------------
                with tc.tile_pool(name="idx", bufs=1) as ipool, \
                     tc.tile_pool(name="psw", bufs=2, space="PSUM") as psw:
                    # y and x processed stacked as [128, 32, 18]
                    # (cols 0:9 = y per tap, 9:18 = x per tap)
                    def it(name):
                        return ipool.tile([128, 32, 18], F32, tag=name, name=name)

                    s_t = it("s")
                    nc.vector.tensor_add(s_t[:], basyx_sb[:], offT[:])

                    t = it("t"); cgt = it("cgt"); f = it("f"); l = it("l")
                    nc.vector.tensor_scalar_add(t[:], s_t[:], MAGIC)
                    nc.vector.tensor_scalar_sub(t[:], t[:], MAGIC)
                    nc.vector.tensor_tensor(cgt[:], t[:], s_t[:], OP.is_gt)
                    nc.vector.tensor_sub(f[:], t[:], cgt[:])
                    nc.vector.tensor_sub(l[:], s_t[:], f[:])

                    c0 = it("c0")
                    nc.vector.tensor_scalar(c0[:], f[:], 0.0, 63.0, OP.max, OP.min)

                    def vmask(src_t, lo, hi, name):
                        a = it(name + "_a"); b = it(name + "_b"); v = it(name + "_v")
                        nc.vector.tensor_scalar(a[:], src_t[:], float(lo), None, OP.is_ge)
                        nc.vector.tensor_scalar(b[:], src_t[:], float(hi), None, OP.is_le)
                        nc.vector.tensor_mul(v[:], a[:], b[:])
                        return v

                    v0 = vmask(f, 0, 63, "v0")
                    c62 = vmask(f, 0, 62, "c62")
                    e_t = it("e")
                    nc.vector.tensor_scalar(e_t[:], f[:], -1.0, None, OP.is_equal)
                    ol = it("ol")
                    nc.vector.tensor_scalar(ol[:], l[:], -1.0, 1.0, OP.mult, OP.add)

                    # Row pair (ida, ida+64) shares one descriptor, so the
                    # y0=-1 case (y1 row = 0 = the ida row itself) moves the
                    # y1 weight onto the y0 slot -- mirror of the x edge trick.
                    # w0 = ol*v0 + l*e  (cols 0:9 = wy0, 9:18 = wx0)
                    # w1 = l*c62        (cols 0:9 = wy1, 9:18 = wx1)
                    w0 = it("w0"); w1 = it("w1"); t3 = it("t3"); t4 = it("t4")
                    nc.vector.tensor_mul(t3[:], ol[:], v0[:])
                    nc.vector.tensor_mul(t4[:], l[:], e_t[:])
                    nc.vector.tensor_add(w0[:], t3[:], t4[:])
                    nc.vector.tensor_mul(w1[:], l[:], c62[:])

                    # w4c[p, qt, k, g, cnr] (bf16) with corners ordered
                    # [(y0,x0), (y0,x1), (y1,x0), (y1,x1)] matching the xt2
                    # column blocks. src: wy from cols 0:9, wx from cols 9:18.
                    for s, (a_t, b_t) in enumerate([(w0, w0), (w0, w1),
                                                    (w1, w0), (w1, w1)]):
                        dst = _mk(w4c[:], s, [list(w4c[:].ap[0]),
                                              [288, NQT], [32, K], [4, 8]])
                        srca = _mk(a_t[:], 0, [list(a_t[:].ap[0]),
                                               [144, NQT], [1, K], [18, 8]])
                        srcb = _mk(b_t[:], 9, [list(b_t[:].ap[0]),
                                               [144, NQT], [1, K], [18, 8]])
                        nc.vector.tensor_tensor(dst, srca, srcb, OP.mult)

                    # gather row index: ida = yc0*64 + xc0 (pixel id, 0..4095)
                    ida = ipool.tile([128, 32, K], F32, tag="ida", name="ida")
                    yc0 = _mk(c0[:], 0, [list(c0[:].ap[0]), [18, 32], [1, K]])
                    xc0 = _mk(c0[:], 9, [list(c0[:].ap[0]), [18, 32], [1, K]])
                    nc.vector.scalar_tensor_tensor(ida[:], yc0, 64.0, xc0,
                                                   OP.mult, OP.add)

                    # idxf[p, k, blk] = ida for pixel blk*128+p, tap k
                    idxf = ipool.tile([128, K, 32], F32, tag="idxf")
                    src_ap = _mk(ida[:], 0, [list(ida[:].ap[0]), [1, K], [K, 32]])
                    nc.vector.tensor_copy(idxf[:], src_ap)

                    # transpose K*32=288 cols in chunks of 128 -> T1, then
                    # scatter 16-row blocks into the SWDGE index layout:
                    # idxw16[j, k, qt, g*8+q] = idx of pixel (qt*8+g)*128+q*16+j
                    T1_sb = ipool.tile([128, 3, 128], F32, tag="T1")
                    nc.vector.memset(T1_sb[:], 0.0)
                    widths = [128, 128, 32]
                    for ch in range(3):
                        wd = widths[ch]
                        ps = psw.tile([128, 128], F32, tag="tpw")
                        in_ap = _mk(idxf[:], ch * 128, [list(idxf[:].ap[0]), [1, wd]])
                        nc.tensor.transpose(ps[0:wd, :], in_ap, idf_sb[:])
                        nc.scalar.copy(T1_sb[0:wd, ch, :], ps[0:wd, :])
                    nc.vector.memset(idxw16[:], 0)
                    for q in range(8):
                        for ch in range(3):
                            wd = widths[ch]
                            ps2 = psw.tile([16, 128], F32, tag="tpw2")
                            in2 = T1_sb[:, ch, q * 16: q * 16 + 16]
                            nc.tensor.transpose(ps2[:], in2, idf_sb[:])
                            # flat col c of ps2: k = ch*4 + c//32,
                            # blk = c%32 = qt*8+g; dst elem offset =
                            # k*256 + qt*64 + g*8 + q
                            base = idxw16[:].offset + ch * 4 * 256 + q
                            pa = list(idxw16[:].ap[0])
                            pa[1] = 16
                            if wd == 128:
                                dims = [pa, [256, 4], [64, 4], [8, 8]]
                            else:
                                dims = [pa, [64, 4], [8, 8]]
                            dst_ap = AP(idxw16[:].tensor, base, dims)
                            nc.vector.tensor_copy(dst_ap, ps2[:, 0:wd])
                    for cgrp in range(8):
                        nc.sync.dma_start(idxw[cgrp * 16:(cgrp + 1) * 16], idxw16[:])

                # ---------------- main: gather, weight, transpose, matmul ----------
                with tc.tile_pool(name="main", bufs=2) as mpool, \
                     tc.tile_pool(name="ybuf", bufs=1) as ypool, \
                     tc.tile_pool(name="pstpp", bufs=4, space="PSUM") as pstpp, \
                     tc.tile_pool(name="psmm", bufs=1, space="PSUM") as psmm:
                    y_sb = ypool.tile([128, 2, HW], F32)
                    s1p = ypool.tile([128, 2, 8], F32, name="s1p") if stage >= 4 else None
                    s2p = ypool.tile([128, 2, 8], F32, name="s2p") if stage >= 4 else None
                    gsrc_ap = _mk(xt2_d, PADROWS * 1024, [[1024, HW], [1, 1024]])

                    for qt in range(NQT):
                        patchT = mpool.tile([128, 18, QPIX], BF16, tag="patchT", bufs=1)
                        psds = {}
                        for chunk in range(2):
                            for oh in range(2):
                                psds[(chunk, oh)] = psmm.tile(
                                    [128, 512], F32, tag=f"mm{chunk}{oh}",
                                    name=f"psd{chunk}{oh}")
                        for k in range(K):
                            ga = mpool.tile([128, 8, 1024], BF16, tag="ga", bufs=4)
                            if stage >= 1:
                                # alternate SWDGE queues so desc-gen of gather
                                # i+1 overlaps the SDMA drain of gather i
                                nc.gpsimd.dma_gather(ga[:], gsrc_ap, idxw[:, k, qt, :],
                                                     QPIX, QPIX, 1024, elem_step=1024,
                                                     queue_num=(qt * K + k) % 2)
                            else:
                                nc.vector.memset(ga[:, 0, 0:16].bitcast(F32), 1.0)

                            p_t = mpool.tile([128, 8, 256], BF16, tag="tp")
                            if stage == 1:
                                nc.vector.tensor_copy(p_t[:, 0, 0:8], ga[:, 0, 0:8])
                            if stage >= 2:
                                aw = mpool.tile([128, 8, 4, 256], BF16, tag="aw",
                                                bufs=1)
                                pa_ga = list(ga[:].ap[0])
                                pa_aw = list(aw[:].ap[0])
                                pa_w = list(w4c[:].ap[0])
                                # aw = ga * w4c (weight broadcast over channels)
                                nc.vector.tensor_tensor(
                                    _mk(aw[:], 0, [pa_aw, [256, 32], [1, 256]]),
                                    _mk(ga[:], 0, [pa_ga, [256, 32], [1, 256]]),
                                    _mk(w4c[:], qt * 288 + k * 32,
                                        [pa_w, [1, 32], [0, 256]]),
                                    OP.mult)
                                # pair sums: (c0+c2, c1+c3) then (+)
                                a2 = mpool.tile([128, 8, 2, 256], BF16, tag="a2")
                                pa_a2 = list(a2[:].ap[0])
                                pa_pt = list(p_t[:].ap[0])
                                nc.vector.tensor_tensor(
                                    _mk(a2[:], 0, [pa_a2, [512, 8], [1, 512]]),
                                    _mk(aw[:], 0, [pa_aw, [1024, 8], [1, 512]]),
                                    _mk(aw[:], 512, [pa_aw, [1024, 8], [1, 512]]),
                                    OP.add)
                                nc.vector.tensor_tensor(
                                    _mk(p_t[:], 0, [pa_pt, [256, 8], [1, 256]]),
                                    _mk(a2[:], 0, [pa_a2, [512, 8], [1, 256]]),
                                    _mk(a2[:], 256, [pa_a2, [512, 8], [1, 256]]),
                                    OP.add)
                            for cb in range(2 if stage >= 3 else 0):
                                for gh in range(2):
                                    psx = pstpp.tile([128, 4, 128], BF16, tag="tpp")
                                    for gi in range(4):
                                        g = gh * 4 + gi
                                        nc.tensor.transpose(
                                            psx[:, gi, :],
                                            p_t[:, g, cb * 128:(cb + 1) * 128],
                                            idb_sb[:])
                                    dst = patchT[:, k * 2 + cb,
                                                 gh * 512:(gh + 1) * 512]
                                    nc.scalar.copy(dst, psx[:])
                            # interleave the main matmul per k: contiguous rhs
                            # from patchT, accumulating into 4 persistent PSUM
                            # banks -- spreads PE work and shrinks the tail
                            if stage >= 4:
                                for cb in range(2):
                                    kc = k * 2 + cb
                                    for oh in range(2):
                                        for chunk in range(2):
                                            nc.tensor.matmul(
                                                psds[(chunk, oh)],
                                                w2_sb[:, kc, oh * 128:(oh + 1) * 128],
                                                patchT[:, kc,
                                                       chunk * 512:(chunk + 1) * 512],
                                                start=(kc == 0), stop=(kc == 17))
                        for chunk in range(2 if stage >= 4 else 0):
                            for oh in range(2):
                                psd = psds[(chunk, oh)]
                                cidx = qt * 2 + chunk
                                nc.scalar.activation(
                                    y_sb[:, oh, qt * 1024 + chunk * 512:
                                         qt * 1024 + (chunk + 1) * 512],
                                    psd, AF.Copy, accum_out=s1p[:, oh, cidx:cidx + 1])
                                sqscr = mpool.tile([128, 512], BF16, tag="sq")
                                nc.scalar.activation(
                                    sqscr[:], psd, AF.Square,
                                    accum_out=s2p[:, oh, cidx:cidx + 1])

                    # ---------------- stats + scale ----------------
                    if stage < 4:
                        nc.vector.memset(y_sb[:, 0, 0:64], 0.0)
                        nc.sync.dma_start(_mk(y_d, 0, [[HW, 128], [1, 64]]),
                                           y_sb[:, 0, 0:64])
                        continue
                    s1 = ypool.tile([128, 2], F32)
                    s2 = ypool.tile([128, 2], F32)
                    nc.vector.reduce_sum(s1[:], s1p[:], axis=mybir.AxisListType.X)
                    nc.vector.reduce_sum(s2[:], s2p[:], axis=mybir.AxisListType.X)
                    mean = ypool.tile([128, 2], F32)
                    nc.vector.tensor_scalar_mul(mean[:], s1[:], 1.0 / HW)
                    ss = ypool.tile([128, 2], F32)
                    nc.vector.tensor_mul(ss[:], s1[:], s1[:])
                    va = ypool.tile([128, 2], F32)
                    vb = ypool.tile([128, 2], F32)
                    var = ypool.tile([128, 2], F32)
                    nc.vector.tensor_scalar_mul(va[:], s2[:], 1.0 / (HW - 1))
                    nc.vector.tensor_scalar_mul(vb[:], ss[:], 1.0 / (HW * (HW - 1.0)))
                    nc.vector.tensor_sub(var[:], va[:], vb[:])
                    nc.vector.tensor_scalar_max(var[:], var[:], 0.0)
                    std = ypool.tile([128, 2], F32)
                    nc.scalar.sqrt(std[:], var[:])
                    arg = ypool.tile([128, 2], F32)
                    nc.vector.tensor_add(arg[:], mean[:], std[:])
                    attn = ypool.tile([128, 2], F32)
                    nc.scalar.activation(attn[:], arg[:], AF.Sigmoid)
                    for oh in range(2):
                        nc.vector.tensor_scalar_mul(y_sb[:, oh, :], y_sb[:, oh, :],
                                                    attn[:, oh:oh + 1])
                        nc.sync.dma_start(
                            _mk(y_d, oh * 128 * HW, [[HW, 128], [1, HW]]),
                            y_sb[:, oh, :])

    nc.compile()
    return nc


def _prep_shared(offset_w, deform_w):
    perm = [2 * i for i in range(9)] + [2 * i + 1 for i in range(9)]
    wp = np.asarray(offset_w, np.float32)[perm]
    wp2 = wp.reshape(18, 2, 128, 9)
    offw = np.ascontiguousarray(wp2.transpose(2, 1, 3, 0)).astype(ml_dtypes.bfloat16)

    wk = np.asarray(deform_w, np.float32).reshape(256, 256, 9)
    t = wk.reshape(256, 2, 128, 9).transpose(2, 3, 1, 0)
    w2 = np.ascontiguousarray(t.reshape(128, 18, 256)).astype(ml_dtypes.bfloat16)

    p = np.arange(128)
    blk = np.arange(32)
    kk = np.arange(9)
    i_pix = blk[None, :, None] * 2 + (p[:, None, None] // 64)
    j_pix = (p[:, None, None] % 64) + 0 * blk[None, :, None]
    basey = np.broadcast_to(
        (i_pix + (kk // 3)[None, None, :] - 1), (128, 32, 9))
    basex = np.broadcast_to(
        (j_pix + (kk % 3)[None, None, :] - 1), (128, 32, 9))
    basyx = np.ascontiguousarray(
        np.concatenate([basey, basex], axis=-1)).astype(np.float32)

    idf = np.eye(128, dtype=np.float32)
    idb = np.eye(128, dtype=np.float32).astype(ml_dtypes.bfloat16)
    return dict(offw=offw, w2=w2, basyx=basyx, idf=idf, idb=idb)


_CACHE = {}


def kernel(x, offset_w, deform_w):
    x = np.asarray(x, np.float32)
    B = x.shape[0]
    assert x.shape == (8, 256, 64, 64)

    if "nc" not in _CACHE:
        _CACHE["nc"] = build_program()
    nc = _CACHE["nc"]

    shared = _prep_shared(offset_w, deform_w)
    in_maps = []
    for b in range(B):
        m = dict(shared)
        m["x"] = np.ascontiguousarray(x[b].reshape(256, HW))
        in_maps.append(m)

    from concourse.bass_utils import run_bass_kernel_spmd
    res = run_bass_kernel_spmd(nc, in_maps, core_ids=list(range(N_CORES)))
    out = np.stack([res.results[b]["y"].reshape(256, 64, 64) for b in range(B)])
    return out.astype(np.float32)


if __name__ == "__main__":
    d = np.load("/root/problem/ref_cache.npz")
    out = kernel(d["x"], d["offset_w"], d["deform_w"])
    err = np.abs(out - d["expected"]).max() / np.abs(d["expected"]).max()
    print("rel err vs cached ref:", err)
